# revision 1
# baseline (speedup 1.0000x reference)
"""3x3 median filter (zero-padded) on TRN2, 8 NeuronCores, exact fp32.

Input  x: (32, 3, 512, 512) float32
Output  : (32, 3, 512, 512) float32, bit-exact vs the jnp sort-based reference.

Strategy
--------
Pure data parallel: batch dim sharded 4-per-core across 8 cores. Per core the
12 images (4 batch x 3 chan) are processed in 3 groups of 4 images x 2
vertical halves of 256 rows.

Exact fp32 median-of-9 via the column-sort decomposition with pair sharing,
15 min/max tensor ops per output element, all on the DVE (the only engine
with 2-input elementwise ops):

  stage 1 (vertical, 5 ops/elem): row-pair tiles O[p]=row r0+2p+1,
    E_sh[p]=row r0+2p+2; their pair min/max (qmn/qmx) is shared by both
    output parities: odd row r0+2p+1 closes its sort3 with E[p]=row r0+2p,
    even row r0+2p+2 with O_sh2[p]=row r0+2p+3. Every DMA is a full
    128-partition transfer (partition-offset / partial-partition DMAs route
    ~75%% of packets through one SDMA engine at ~23 GB/s) -- the two
    vertical halves overlap by two rows to make that possible. Image rows
    0 and 511 (windows contain the zero pad row) are handled by one tiny
    24-partition pass batched across all images.

  stage 2 (horizontal, 10 ops/elem): zero-padded width-514 (min, med, max)
    fields; column pair-sharing at even columns; med9 = med3(max3(mins),
    med3(meds), min3(maxes)).

All W shifts are strided free-dim APs (measured: same DVE cost as dense).
Vertical halo comes from extra strided HBM loads (reads x2, hidden under
compute). Loads split across the two HWDGE queues (SP + ACT); stores go to
the GpSimd SWDGE queue so they never block a later block's loads.

Measured: 433 us HW exec per core (DVE ~98%% busy; elementwise floor for
15 fp32 ops/elem at the DVE's 1x fp32 rate is ~388 us), bit-exact output.
"""
import sys

if "/opt/trn_rl_repo" not in sys.path:
    sys.path.insert(0, "/opt/trn_rl_repo")

import numpy as np
import concourse.bacc as bacc
import concourse.mybir as mybir
import concourse.tile as tile
from concourse import bass_utils

B, C, H, W = 32, 3, 512, 512
N_CORES = 8
B_PER = B // N_CORES          # 4 batches per core
NIMG = B_PER * C              # 12 images per core
GIMG = 4                      # images per tile group
FW = GIMG * W                 # free width of row tiles
PW = W + 2                    # padded per-image width (514)
FP = GIMG * PW                # free width of padded tiles
HH = H // 2                   # 256 rows per vertical half
P = 128                       # partitions = row pairs per half

F32 = mybir.dt.float32
MIN = mybir.AluOpType.min
MAX = mybir.AluOpType.max

_PROGRAM = None


def _stage2(nc, pm, PMN, PMD, PMX, OUT, npart, nimg, out_np=None):
    """Horizontal pass: padded (min, med, max) fields [npart, nimg*514] ->
    median into OUT [npart, nimg*512] (interleaved columns).
    out_np: partition count for the final output writes (default npart)."""
    v = lambda T: T[:].rearrange("p (i w) -> p i w", w=PW)[0:npart, 0:nimg]
    mn, md, mx = v(PMN), v(PMD), v(PMX)

    def t2(tag, fw):
        return pm.tile([P, GIMG * fw], F32, tag=tag, name=tag)

    def tv(T, fw):
        return T[:].rearrange("p (i w) -> p i w", w=fw)[0:npart, 0:nimg]

    NP = PW // 2   # 257 pairs per image
    HWW = W // 2   # 256 outputs per column parity
    U = t2("U", NP); Vt = t2("V", NP); Qmn = t2("Qmn", NP); Qmx = t2("Qmx", NP)
    Uv, Vv, Qmnv, Qmxv = tv(U, NP), tv(Vt, NP), tv(Qmn, NP), tv(Qmx, NP)

    # pairs over padded columns (2k, 2k+1)
    nc.vector.tensor_tensor(Uv, mn[:, :, 0:PW:2], mn[:, :, 1:PW:2], op=MAX)
    nc.vector.tensor_tensor(Vv, mx[:, :, 0:PW:2], mx[:, :, 1:PW:2], op=MIN)
    nc.vector.tensor_tensor(Qmnv, md[:, :, 0:PW:2], md[:, :, 1:PW:2], op=MIN)
    nc.vector.tensor_tensor(Qmxv, md[:, :, 0:PW:2], md[:, :, 1:PW:2], op=MAX)

    # merged half tiles: layout [P, (h, i, m)], h = column parity (0=even w)
    AA = t2("AA", 2 * HWW); CC = t2("CC", 2 * HWW)
    TB = t2("TB", 2 * HWW); BB = t2("BB", 2 * HWW)
    MX1 = t2("MX1", 2 * HWW)
    MN1 = pm.tile([P, GIMG * 2 * HWW], F32, tag="TB", name="MN1")  # alias: TB dead
    TF = pm.tile([P, GIMG * 2 * HWW], F32, tag="AA", name="TF")  # alias: AA dead

    def hcat(T):  # [npart, 2, nimg, HWW]
        return T[:].rearrange("p (h i m) -> p h i m", h=2, i=GIMG)[
            0:npart, :, 0:nimg
        ]

    def hv(T, h):  # [npart, nimg, HWW]
        return hcat(T)[:, h]

    # even output columns w=2m: pair k=m + third padded col 2m+2
    nc.vector.tensor_tensor(hv(AA, 0), Uv[:, :, 0:HWW], mn[:, :, 2:PW:2], op=MAX)
    nc.vector.tensor_tensor(hv(CC, 0), Vv[:, :, 0:HWW], mx[:, :, 2:PW:2], op=MIN)
    nc.vector.tensor_tensor(hv(TB, 0), Qmxv[:, :, 0:HWW], md[:, :, 2:PW:2], op=MIN)
    nc.vector.tensor_tensor(hv(BB, 0), Qmnv[:, :, 0:HWW], hv(TB, 0), op=MAX)
    # odd output columns w=2m+1: pair k=m+1 + third padded col 2m+1
    nc.vector.tensor_tensor(hv(AA, 1), Uv[:, :, 1 : HWW + 1], mn[:, :, 1 : PW - 1 : 2], op=MAX)
    nc.vector.tensor_tensor(hv(CC, 1), Vv[:, :, 1 : HWW + 1], mx[:, :, 1 : PW - 1 : 2], op=MIN)
    nc.vector.tensor_tensor(hv(TB, 1), Qmxv[:, :, 1 : HWW + 1], md[:, :, 1 : PW - 1 : 2], op=MIN)
    nc.vector.tensor_tensor(hv(BB, 1), Qmnv[:, :, 1 : HWW + 1], hv(TB, 1), op=MAX)

    # final med3(A, B, C), both parities in single full-width ops; the last
    # op writes straight into OUT via a parity-interleaving 4D AP
    nc.vector.tensor_tensor(hcat(MN1), hcat(AA), hcat(BB), op=MIN)
    nc.vector.tensor_tensor(hcat(MX1), hcat(AA), hcat(BB), op=MAX)
    nc.vector.tensor_tensor(hcat(TF), hcat(MX1), hcat(CC), op=MIN)
    ovm = OUT[:].rearrange("p (i m h) -> p h i m", h=2, m=HWW)[0:npart, :, 0:nimg]
    nc.vector.tensor_tensor(ovm, hcat(MN1), hcat(TF), op=MAX)


def _alloc_padded(nc, pm, names, npart, tags=None):
    padded = {}
    for j, name in enumerate(names):
        T = pm.tile([P, FP], F32, tag=(tags[j] if tags else name), name=name)
        Tv = T[:].rearrange("p (i w) -> p i w", w=PW)
        # zero the two pad columns (0 and 513) of each image segment
        # (on GpSimd: it is otherwise idle, and this keeps the DVE stream pure)
        nc.gpsimd.memset(Tv[0:npart, :, 0 : PW : PW - 1], 0.0)
        padded[name] = T
    return padded


def _block(nc, pio, pm, xh, oh, g, half):
    """One vertical half of one image group: covers odd output rows
    r0+1 .. r0+255 and even rows r0+2 .. r0+256. The two halves (r0 = 0 and
    254) overlap by two rows so that every DMA is a full 128-partition
    transfer of in-bounds rows (non-128-partition DMAs route ~75%% of their
    packets through one SDMA engine at ~25 GB/s). Rows 0 and 511 are done
    by _edge_rows_pass."""
    r0 = 0 if half == 0 else H - HH - 2
    i0 = GIMG * g

    E = pio.tile([P, FW], F32, tag="E", name="E")
    O = pio.tile([P, FW], F32, tag="O", name="O")
    E_sh = pio.tile([P, FW], F32, tag="E_sh", name="E_sh")
    O_sh2 = pio.tile([P, FW], F32, tag="O_sh2", name="O_sh2")

    img = lambda r_lo: xh[r_lo : min(r_lo + 2 * P, H) : 2, i0 : i0 + GIMG, :]
    # queue order matters (HWDGE queues are FIFOs): the (O, E_sh) pair feeds
    # the first op of the block, so those loads go first on each queue
    nc.sync.dma_start(E_sh[:], img(r0 + 2))     # rows r0+2p+2
    nc.scalar.dma_start(O[:], img(r0 + 1))      # rows r0+2p+1
    nc.sync.dma_start(E[:], img(r0))            # rows r0+2p
    nc.scalar.dma_start(O_sh2[:], img(r0 + 3))  # rows r0+2p+3

    # stage 1: shared pair = (O, E_sh) = rows (2p+1, 2p+2)
    qmn = pm.tile([P, FW], F32, tag="qmn", name="qmn", bufs=2)
    qmx = pm.tile([P, FW], F32, tag="qmx", name="qmx", bufs=2)
    nc.vector.tensor_tensor(qmn[:], O[:], E_sh[:], op=MIN)
    nc.vector.tensor_tensor(qmx[:], O[:], E_sh[:], op=MAX)

    padded = _alloc_padded(
        nc, pm, ("MN_e", "MD_e", "MX_e", "MN_o", "MD_o", "MX_o"), P
    )
    dv = lambda T: T[:].rearrange("p (i w) -> p i w", w=PW)[:, :, 1 : W + 1]
    wv = lambda T: T[:].rearrange("p (i w) -> p i w", w=W)
    # stage-1 temps alias stage-2 slots (disjoint lifetimes)
    t_o = pm.tile([P, FW], F32, tag="CC", name="t_o")
    t_e = pm.tile([P, FW], F32, tag="TB", name="t_e")

    # odd output rows r0+2p+1: pair + E (row r0+2p)
    nc.vector.tensor_tensor(dv(padded["MN_o"]), wv(qmn), wv(E), op=MIN)
    nc.vector.tensor_tensor(dv(padded["MX_o"]), wv(qmx), wv(E), op=MAX)
    nc.vector.tensor_tensor(wv(t_o), wv(qmx), wv(E), op=MIN)
    nc.vector.tensor_tensor(dv(padded["MD_o"]), wv(qmn), wv(t_o), op=MAX)
    # even output rows r0+2p+2: pair + O_sh2 (row r0+2p+3)
    nc.vector.tensor_tensor(dv(padded["MN_e"]), wv(qmn), wv(O_sh2), op=MIN)
    nc.vector.tensor_tensor(dv(padded["MX_e"]), wv(qmx), wv(O_sh2), op=MAX)
    nc.vector.tensor_tensor(wv(t_e), wv(qmx), wv(O_sh2), op=MIN)
    nc.vector.tensor_tensor(dv(padded["MD_e"]), wv(qmn), wv(t_e), op=MAX)

    OUT_e = pio.tile([P, FW], F32, tag="OUT_e", name="OUT_e")
    OUT_o = pio.tile([P, FW], F32, tag="OUT_o", name="OUT_o")
    _stage2(nc, pm, padded["MN_o"], padded["MD_o"], padded["MX_o"], OUT_o,
            P, GIMG)
    _stage2(nc, pm, padded["MN_e"], padded["MD_e"], padded["MX_e"], OUT_e,
            P, GIMG)

    out_img = lambda r_lo: oh[r_lo : min(r_lo + 2 * P, H) : 2, i0 : i0 + GIMG, :]
    # stores go to the SWDGE queue: HWDGE queues are FIFOs, so a store
    # parked on a load queue would block the next block's loads
    nc.gpsimd.dma_start(out_img(r0 + 1), OUT_o[:])
    nc.gpsimd.dma_start(out_img(r0 + 2), OUT_e[:])


def _edge_rows_pass(nc, pio, pm, xi, oi):
    """Image rows 0 and 511 for all 12 images (windows contain the zero pad
    row). 24-partition tiles: p 0..11 = row 0 of image p (partner row 1);
    p 12..23 = row 511 of image p-12 (partner row 510).
    xi/oi: [12, 512, 512] (image-major) DRAM views."""
    NE = 2 * NIMG
    R0 = pio.tile([NE, W], F32, tag="R0", name="R0")   # the edge row itself
    R1 = pio.tile([NE, W], F32, tag="R1", name="R1")   # its interior neighbor
    nc.sync.dma_start(R0[0:NIMG, :], xi[:, 0, :])
    nc.scalar.dma_start(R1[0:NIMG, :], xi[:, 1, :])
    nc.sync.dma_start(R0[NIMG:NE, :], xi[:, H - 1, :])
    nc.scalar.dma_start(R1[NIMG:NE, :], xi[:, H - 2, :])

    rmn = pm.tile([NE, W], F32, tag="qmn", name="rmn", bufs=2)
    rmx = pm.tile([NE, W], F32, tag="qmx", name="rmx", bufs=2)
    nc.vector.tensor_tensor(rmn[:], R0[:], R1[:], op=MIN)
    nc.vector.tensor_tensor(rmx[:], R0[:], R1[:], op=MAX)

    padded = _alloc_padded(
        nc, pm, ("MN_0", "MD_0", "MX_0"), NE, tags=("MN_e", "MD_e", "MX_e")
    )
    dv = lambda T: T[:].rearrange("p (i w) -> p i w", w=PW)[0:NE, 0:1, 1 : W + 1]
    w1 = lambda T: T[:].rearrange("p (i w) -> p i w", i=1)
    # sort3 with the zero pad row: min/max vs 0.0, med = max(mn, min(mx, 0))
    nc.vector.tensor_scalar_min(dv(padded["MN_0"]), w1(rmn), 0.0)
    nc.vector.tensor_scalar_max(dv(padded["MX_0"]), w1(rmx), 0.0)
    nc.vector.scalar_tensor_tensor(
        dv(padded["MD_0"]), w1(rmx), 0.0, w1(rmn), op0=MIN, op1=MAX
    )

    OUT0 = pio.tile([NE, W], F32, tag="OUT0", name="OUT0")
    _stage2(nc, pm, padded["MN_0"], padded["MD_0"], padded["MX_0"], OUT0,
            NE, 1)
    ov = OUT0[:].rearrange("p (i w) -> p i w", w=W)
    nc.gpsimd.dma_start(oi[:, 0, :], ov[0:NIMG])
    nc.gpsimd.dma_start(oi[:, H - 1, :], ov[NIMG:NE])


def build_program():
    nc = bacc.Bacc(
        "TRN2", target_bir_lowering=False, debug=False, num_devices=N_CORES
    )
    x_d = nc.dram_tensor("x", [B_PER, C, H, W], F32, kind="ExternalInput").ap()
    o_d = nc.dram_tensor("out", [B_PER, C, H, W], F32, kind="ExternalOutput").ap()
    xh = x_d.rearrange("b c h w -> h (b c) w")  # [512, 12, 512]
    oh = o_d.rearrange("b c h w -> h (b c) w")
    xi = x_d.rearrange("b c h w -> (b c) h w")  # [12, 512, 512]
    oi = o_d.rearrange("b c h w -> (b c) h w")

    with tile.TileContext(nc) as tc:
        with (
            tc.tile_pool(name="io", bufs=1) as pio,
            tc.tile_pool(name="mid", bufs=1) as pm,
        ):
            _edge_rows_pass(nc, pio, pm, xi, oi)
            for g in range(NIMG // GIMG):
                for half in range(2):
                    _block(nc, pio, pm, xh, oh, g, half)
    nc.compile()
    return nc


def _get_program():
    global _PROGRAM
    if _PROGRAM is None:
        _PROGRAM = build_program()
    return _PROGRAM


def kernel(**inputs) -> np.ndarray:
    x = np.ascontiguousarray(np.asarray(inputs["x"], dtype=np.float32))
    assert x.shape == (B, C, H, W), x.shape
    nc = _get_program()
    in_maps = [{"x": x[k * B_PER : (k + 1) * B_PER]} for k in range(N_CORES)]
    res = bass_utils.run_bass_kernel_spmd(nc, in_maps, core_ids=list(range(N_CORES)))
    return np.concatenate([res.results[k]["out"] for k in range(N_CORES)], axis=0)



# revision 6
# speedup vs baseline: 1.6304x; 1.6304x over previous
"""3x3 median filter (zero-padded) on TRN2, 8 NeuronCores, bf16 datapath.

Input  x: (32, 3, 512, 512) float32
Output  : (32, 3, 512, 512) float32.

Accuracy: the median network only ever SELECTS one of its 9 inputs (min/max
ops create no new values), so the device-side bf16 result equals
bf16(round(x)) of the element that is the median of the rounded window.
Order statistics are 1-Lipschitz under sup-norm perturbation, so the end to
end error is <= 2^-8 relative -- far inside the 2e-2 gate.

Strategy
--------
Pure data parallel: batch dim sharded 4-per-core across 8 cores; per core
12 images (4 batch x 3 chan) in 3 groups of 4 images x 2 vertical halves.

bf16 doubles DVE tensor_tensor throughput (2x_1P perf mode) but ONLY for
unit-stride, 4-byte-aligned access patterns, so the horizontal stage is
restructured from the fp32 baseline's stride-2 parity tricks into dense
shifted-field form:

  stage 1 (vertical, 5 TT/elem, all dense): identical to baseline --
  row-pair (qmn,qmx) shared by both output-row parities.

  stage 2 (horizontal, 12 TT/elem, all dense + aligned): per field
  F in {MN,MD,MX} build s1F[j]=F[j+1] (the ONLY odd-element shift, done
  as a ScalarE copy -- ACT is otherwise idle), then
     P[j]  = op(F[j], s1F[j])          # aligned TT, 2x
     R[j]  = op(P[j], F[j+2])          # +2 elems = 4B-aligned shift, 2x
  R[j] = sliding-3 result centered at col j+1; the final med3(Rmn,Rmd,Rmx)
  writes straight into an OUT grid whose per-image segment holds col c at
  position c+1, so the wide write starts at even offset 2 and the DMA
  store (which doesn't care about alignment) un-shifts.

  Output cols 0 and 511 (windows containing the zero pad column) are a
  handful of width-2-per-image boundary ops reusing P/Qmn/Qmx at grid
  positions {0, 510}.

Grid: per-image segment width 514 (even -> every segment start keeps
4B parity). Segment positions 512..513 are scratch: stage-1 ops run flat
over the whole grid and compute garbage there; no stored output ever
reads a garbage lane (R[510] is garbage but cols 0/511 come from the
boundary ops which overwrite OUT positions 1 and 512).

Image rows 0 and 511 (windows contain the zero pad row) are one small
24-partition pass batched across all images, same pipeline.

Engine budget per core (theory): DVE 17 TT/elem at 2x ~= 218us, ACT
3 copies/elem ~= 61us, DMA ~19MB ~= well under. Loads on the SP+ACT
HWDGE queues, stores on the GpSimd SWDGE queue (baseline-proven split).
"""
import sys

if "/opt/trn_rl_repo" not in sys.path:
    sys.path.insert(0, "/opt/trn_rl_repo")

import numpy as np
import ml_dtypes
import concourse.bacc as bacc
import concourse.mybir as mybir
import concourse.tile as tile
from concourse import bass_utils

B, C, H, W = 32, 3, 512, 512
N_CORES = 8
B_PER = B // N_CORES          # 4 batches per core
NIMG = B_PER * C              # 12 images per core
GIMG = 4                      # images per tile group
PW = W + 2                    # per-image grid width (514, even)
FP = GIMG * PW                # flat grid width of row tiles (2056)
HH = H // 2                   # 256 rows per vertical half
P = 128                       # partitions = row pairs per half
NE = 2 * NIMG                 # partitions in the edge-rows pass (24)

BF16 = mybir.dt.bfloat16
MIN = mybir.AluOpType.min
MAX = mybir.AluOpType.max

_PROGRAM = None


def _seg(T, npart, nimg):
    """[npart, nimg, 514] per-image-segment view."""
    return T[:].rearrange("p (i w) -> p i w", w=PW)[0:npart, 0:nimg]


def _stage2_copies(nc, pm, MN, MD, MX, npart, nimg, pref):
    """ScalarE: the one odd-element shift per field, s1F[j] = F[j+1].
    Issued early so ACT overlaps the DVE work of the previous parity."""
    NF = nimg * PW
    s1 = {}
    for name, F in (("MN", MN), ("MD", MD), ("MX", MX)):
        T = pm.tile([P, FP], BF16, tag=f"s1{name}", name=f"{pref}s1{name}", bufs=2)
        nc.scalar.copy(T[:][0:npart, 0 : NF - 1], F[:][0:npart, 1:NF])
        s1[name] = T
    return s1


def _stage2_compute(nc, pm, MN, MD, MX, s1, OUT, npart, nimg, pref):
    """DVE: dense aligned sliding-3 per field + final med3 -> OUT grid.
    OUT per-image position c+1 holds output col c."""
    NF = nimg * PW
    NI = NF - 2
    fl = lambda T, a, b: T[:][0:npart, a:b]

    def t2(tag):
        return pm.tile([P, FP], BF16, tag=tag, name=f"{pref}{tag}")

    Pmn, Pmx, Qmn, Qmx = t2("Pmn"), t2("Pmx"), t2("Qmn"), t2("Qmx")
    tmd, Rmn, Rmd, Rmx = t2("tmd"), t2("Rmn"), t2("Rmd"), t2("Rmx")
    MN1, MX1, TF = t2("MN1"), t2("MX1"), t2("TF")

    tt = nc.vector.tensor_tensor
    # adjacent-column pairs (j, j+1)
    tt(fl(Pmn, 0, NI), fl(MN, 0, NI), fl(s1["MN"], 0, NI), op=MAX)
    tt(fl(Qmn, 0, NI), fl(MD, 0, NI), fl(s1["MD"], 0, NI), op=MIN)
    tt(fl(Qmx, 0, NI), fl(MD, 0, NI), fl(s1["MD"], 0, NI), op=MAX)
    tt(fl(Pmx, 0, NI), fl(MX, 0, NI), fl(s1["MX"], 0, NI), op=MIN)
    # close the window with the +2 (aligned) shift: R[j] ~ output col j+1
    tt(fl(Rmn, 0, NI), fl(Pmn, 0, NI), fl(MN, 2, NF), op=MAX)
    tt(fl(tmd, 0, NI), fl(Qmx, 0, NI), fl(MD, 2, NF), op=MIN)
    tt(fl(Rmd, 0, NI), fl(Qmn, 0, NI), fl(tmd, 0, NI), op=MAX)
    tt(fl(Rmx, 0, NI), fl(Pmx, 0, NI), fl(MX, 2, NF), op=MIN)
    # med3(Rmn, Rmd, Rmx); last op writes output cols 1..510 at grid
    # positions 2..511 (even start -> stays in 2x mode)
    tt(fl(MN1, 0, NI), fl(Rmn, 0, NI), fl(Rmd, 0, NI), op=MIN)
    tt(fl(MX1, 0, NI), fl(Rmn, 0, NI), fl(Rmd, 0, NI), op=MAX)
    tt(fl(TF, 0, NI), fl(MX1, 0, NI), fl(Rmx, 0, NI), op=MIN)
    ov = _seg(OUT, npart, nimg)[:, :, 2:512]
    m1v = _seg(MN1, npart, nimg)[:, :, 0:510]
    tfv = _seg(TF, npart, nimg)[:, :, 0:510]
    tt(ov, m1v, tfv, op=MAX)

    # boundary cols 0 and 511: window contains the zero pad column, so
    # A=max(P,0), C=min(Pmx,0), B=max(Qmn,min(Qmx,0)) at grid pos {0,510},
    # then med3 -> OUT positions {1, 512}. Width 2*nimg per op.
    gv = lambda T: _seg(T, npart, nimg)[:, :, 0:511:510]
    bt = lambda tag: pm.tile([P, 2 * GIMG], BF16, tag=tag, name=f"{pref}{tag}")
    bv = lambda T: T[:].rearrange("p (i c) -> p i c", c=2)[0:npart, 0:nimg]
    BA, BB, BC = bt("BA"), bt("BB"), bt("BC")
    B1, B2, B3 = bt("B1"), bt("B2"), bt("B3")
    nc.vector.tensor_scalar_max(bv(BA), gv(Pmn), 0.0)
    nc.vector.tensor_scalar_min(bv(BC), gv(Pmx), 0.0)
    nc.vector.scalar_tensor_tensor(bv(BB), gv(Qmx), 0.0, gv(Qmn), op0=MIN, op1=MAX)
    tt(bv(B1), bv(BA), bv(BB), op=MIN)
    tt(bv(B2), bv(BA), bv(BB), op=MAX)
    tt(bv(B3), bv(B2), bv(BC), op=MIN)
    obv = _seg(OUT, npart, nimg)[:, :, 1:513:511]
    tt(obv, bv(B1), bv(B3), op=MAX)


def _block(nc, pio, pm, xh, oh, g, half):
    """One vertical half of one image group: odd output rows r0+1..r0+255,
    even rows r0+2..r0+256 (halves overlap by two rows so every DMA is a
    full 128-partition transfer). Rows 0 and 511 via _edge_rows_pass."""
    r0 = 0 if half == 0 else H - HH - 2
    i0 = GIMG * g

    E = pio.tile([P, FP], BF16, tag="E", name="E", bufs=2)
    O = pio.tile([P, FP], BF16, tag="O", name="O", bufs=2)
    E_sh = pio.tile([P, FP], BF16, tag="E_sh", name="E_sh", bufs=2)
    O_sh2 = pio.tile([P, FP], BF16, tag="O_sh2", name="O_sh2", bufs=2)

    # scratch cols 512..513 of each segment are read (flat stage-1 ops)
    # but never loaded: define them once per buffer so no uninit reads
    for T in (E, O, E_sh, O_sh2):
        nc.gpsimd.memset(_seg(T, P, GIMG)[:, :, W:PW], 0.0)

    lv = lambda T: _seg(T, P, GIMG)[:, :, 0:W]
    img = lambda r_lo: xh[r_lo : min(r_lo + 2 * P, H) : 2, i0 : i0 + GIMG, :]
    # queue order matters (HWDGE queues are FIFOs): the (O, E_sh) pair
    # feeds the first op of the block, so those loads go first
    nc.sync.dma_start(lv(E_sh), img(r0 + 2))     # rows r0+2p+2
    nc.scalar.dma_start(lv(O), img(r0 + 1))      # rows r0+2p+1
    nc.sync.dma_start(lv(E), img(r0))            # rows r0+2p
    nc.scalar.dma_start(lv(O_sh2), img(r0 + 3))  # rows r0+2p+3

    # stage 1: shared pair = (O, E_sh) = rows (2p+1, 2p+2); all ops flat
    # over the whole grid (scratch cols compute garbage, never stored)
    qmn = pm.tile([P, FP], BF16, tag="qmn", name="qmn", bufs=2)
    qmx = pm.tile([P, FP], BF16, tag="qmx", name="qmx", bufs=2)
    tt = nc.vector.tensor_tensor
    tt(qmn[:], O[:], E_sh[:], op=MIN)
    tt(qmx[:], O[:], E_sh[:], op=MAX)

    def fld(tag):
        return pm.tile([P, FP], BF16, tag=tag, name=tag)

    MN_o, MD_o, MX_o = fld("MN_o"), fld("MD_o"), fld("MX_o")
    MN_e, MD_e, MX_e = fld("MN_e"), fld("MD_e"), fld("MX_e")
    t_o, t_e = fld("t_o"), fld("t_e")

    # odd output rows r0+2p+1: pair + E (row r0+2p)
    tt(MN_o[:], qmn[:], E[:], op=MIN)
    tt(MX_o[:], qmx[:], E[:], op=MAX)
    tt(t_o[:], qmx[:], E[:], op=MIN)
    tt(MD_o[:], qmn[:], t_o[:], op=MAX)
    # even output rows r0+2p+2: pair + O_sh2 (row r0+2p+3)
    tt(MN_e[:], qmn[:], O_sh2[:], op=MIN)
    tt(MX_e[:], qmx[:], O_sh2[:], op=MAX)
    tt(t_e[:], qmx[:], O_sh2[:], op=MIN)
    tt(MD_e[:], qmn[:], t_e[:], op=MAX)

    # ACT shifts for both parities up front -> overlap with DVE stage 2
    s1_o = _stage2_copies(nc, pm, MN_o, MD_o, MX_o, P, GIMG, "o_")
    s1_e = _stage2_copies(nc, pm, MN_e, MD_e, MX_e, P, GIMG, "e_")

    OUT_o = pio.tile([P, FP], BF16, tag="OUT_o", name="OUT_o", bufs=2)
    OUT_e = pio.tile([P, FP], BF16, tag="OUT_e", name="OUT_e", bufs=2)
    _stage2_compute(nc, pm, MN_o, MD_o, MX_o, s1_o, OUT_o, P, GIMG, "o_")
    _stage2_compute(nc, pm, MN_e, MD_e, MX_e, s1_e, OUT_e, P, GIMG, "e_")

    out_img = lambda r_lo: oh[r_lo : min(r_lo + 2 * P, H) : 2, i0 : i0 + GIMG, :]
    ostore = lambda T: _seg(T, P, GIMG)[:, :, 1:513]
    # stores on the SWDGE queue so they never block a later block's loads
    nc.gpsimd.dma_start(out_img(r0 + 1), ostore(OUT_o))
    nc.gpsimd.dma_start(out_img(r0 + 2), ostore(OUT_e))


def _edge_rows_pass(nc, pio, pm, xi, oi):
    """Image rows 0 and 511 for all 12 images (windows contain the zero pad
    row). 24-partition tiles: p 0..11 = row 0 of image p (partner row 1);
    p 12..23 = row 511 of image p-12 (partner row 510)."""
    R0 = pio.tile([NE, PW], BF16, tag="R0", name="R0")
    R1 = pio.tile([NE, PW], BF16, tag="R1", name="R1")
    for T in (R0, R1):
        nc.gpsimd.memset(T[:][0:NE, W:PW], 0.0)
    nc.sync.dma_start(R0[:][0:NIMG, 0:W], xi[:, 0, :])
    nc.scalar.dma_start(R1[:][0:NIMG, 0:W], xi[:, 1, :])
    nc.sync.dma_start(R0[:][NIMG:NE, 0:W], xi[:, H - 1, :])
    nc.scalar.dma_start(R1[:][NIMG:NE, 0:W], xi[:, H - 2, :])

    rmn = pm.tile([NE, PW], BF16, tag="rmn", name="rmn")
    rmx = pm.tile([NE, PW], BF16, tag="rmx", name="rmx")
    nc.vector.tensor_tensor(rmn[:], R0[:], R1[:], op=MIN)
    nc.vector.tensor_tensor(rmx[:], R0[:], R1[:], op=MAX)

    # vertical sort3 with the zero pad row: min/max vs 0, med=max(mn,min(mx,0))
    MN0 = pm.tile([NE, PW], BF16, tag="MN_0", name="MN_0")
    MD0 = pm.tile([NE, PW], BF16, tag="MD_0", name="MD_0")
    MX0 = pm.tile([NE, PW], BF16, tag="MX_0", name="MX_0")
    nc.vector.tensor_scalar_min(MN0[:], rmn[:], 0.0)
    nc.vector.tensor_scalar_max(MX0[:], rmx[:], 0.0)
    nc.vector.scalar_tensor_tensor(MD0[:], rmx[:], 0.0, rmn[:], op0=MIN, op1=MAX)

    NF = PW
    s1 = {}
    for name, F in (("MN", MN0), ("MD", MD0), ("MX", MX0)):
        T = pm.tile([NE, PW], BF16, tag=f"z1{name}", name=f"z1{name}")
        nc.scalar.copy(T[:][0:NE, 0 : NF - 1], F[:][0:NE, 1:NF])
        s1[name] = T

    OUT0 = pio.tile([NE, PW], BF16, tag="OUT0", name="OUT0")
    _stage2_compute_small(nc, pm, MN0, MD0, MX0, s1, OUT0)
    nc.gpsimd.dma_start(oi[:, 0, :], OUT0[:][0:NIMG, 1:513])
    nc.gpsimd.dma_start(oi[:, H - 1, :], OUT0[:][NIMG:NE, 1:513])


def _stage2_compute_small(nc, pm, MN, MD, MX, s1, OUT):
    """Same dataflow as _stage2_compute on [NE, PW] tiles (nimg=1)."""
    NF = PW
    NI = NF - 2
    fl = lambda T, a, b: T[:][0:NE, a:b]

    def t2(tag):
        return pm.tile([NE, PW], BF16, tag=f"z{tag}", name=f"z{tag}")

    Pmn, Pmx, Qmn, Qmx = t2("Pmn"), t2("Pmx"), t2("Qmn"), t2("Qmx")
    tmd, Rmn, Rmd, Rmx = t2("tmd"), t2("Rmn"), t2("Rmd"), t2("Rmx")
    MN1, MX1, TF = t2("MN1"), t2("MX1"), t2("TF")

    tt = nc.vector.tensor_tensor
    tt(fl(Pmn, 0, NI), fl(MN, 0, NI), fl(s1["MN"], 0, NI), op=MAX)
    tt(fl(Qmn, 0, NI), fl(MD, 0, NI), fl(s1["MD"], 0, NI), op=MIN)
    tt(fl(Qmx, 0, NI), fl(MD, 0, NI), fl(s1["MD"], 0, NI), op=MAX)
    tt(fl(Pmx, 0, NI), fl(MX, 0, NI), fl(s1["MX"], 0, NI), op=MIN)
    tt(fl(Rmn, 0, NI), fl(Pmn, 0, NI), fl(MN, 2, NF), op=MAX)
    tt(fl(tmd, 0, NI), fl(Qmx, 0, NI), fl(MD, 2, NF), op=MIN)
    tt(fl(Rmd, 0, NI), fl(Qmn, 0, NI), fl(tmd, 0, NI), op=MAX)
    tt(fl(Rmx, 0, NI), fl(Pmx, 0, NI), fl(MX, 2, NF), op=MIN)
    tt(fl(MN1, 0, NI), fl(Rmn, 0, NI), fl(Rmd, 0, NI), op=MIN)
    tt(fl(MX1, 0, NI), fl(Rmn, 0, NI), fl(Rmd, 0, NI), op=MAX)
    tt(fl(TF, 0, NI), fl(MX1, 0, NI), fl(Rmx, 0, NI), op=MIN)
    tt(fl(OUT, 2, 512), fl(MN1, 0, 510), fl(TF, 0, 510), op=MAX)

    gv = lambda T: T[:][0:NE, 0:511:510]
    bt = lambda tag: pm.tile([NE, 2], BF16, tag=f"z{tag}b", name=f"z{tag}b")
    BA, BB, BC = bt("BA"), bt("BB"), bt("BC")
    B1, B2, B3 = bt("B1"), bt("B2"), bt("B3")
    nc.vector.tensor_scalar_max(BA[:], gv(Pmn), 0.0)
    nc.vector.tensor_scalar_min(BC[:], gv(Pmx), 0.0)
    nc.vector.scalar_tensor_tensor(BB[:], gv(Qmx), 0.0, gv(Qmn), op0=MIN, op1=MAX)
    tt(B1[:], BA[:], BB[:], op=MIN)
    tt(B2[:], BA[:], BB[:], op=MAX)
    tt(B3[:], B2[:], BC[:], op=MIN)
    tt(OUT[:][0:NE, 1:513:511], B1[:], B3[:], op=MAX)


def build_program():
    nc = bacc.Bacc(
        "TRN2", target_bir_lowering=False, debug=False, num_devices=N_CORES
    )
    x_d = nc.dram_tensor("x", [B_PER, C, H, W], BF16, kind="ExternalInput").ap()
    o_d = nc.dram_tensor("out", [B_PER, C, H, W], BF16, kind="ExternalOutput").ap()
    xh = x_d.rearrange("b c h w -> h (b c) w")  # [512, 12, 512]
    oh = o_d.rearrange("b c h w -> h (b c) w")
    xi = x_d.rearrange("b c h w -> (b c) h w")  # [12, 512, 512]
    oi = o_d.rearrange("b c h w -> (b c) h w")

    with tile.TileContext(nc) as tc:
        with (
            tc.tile_pool(name="io", bufs=1) as pio,
            tc.tile_pool(name="mid", bufs=1) as pm,
        ):
            _edge_rows_pass(nc, pio, pm, xi, oi)
            for g in range(NIMG // GIMG):
                for half in range(2):
                    _block(nc, pio, pm, xh, oh, g, half)
    nc.compile()
    return nc


def _get_program():
    global _PROGRAM
    if _PROGRAM is None:
        _PROGRAM = build_program()
    return _PROGRAM


def make_in_maps(x: np.ndarray):
    xb = np.ascontiguousarray(x).astype(ml_dtypes.bfloat16)
    return [{"x": xb[k * B_PER : (k + 1) * B_PER]} for k in range(N_CORES)]


def kernel(**inputs) -> np.ndarray:
    x = np.asarray(inputs["x"], dtype=np.float32)
    assert x.shape == (B, C, H, W), x.shape
    nc = _get_program()
    res = bass_utils.run_bass_kernel_spmd(
        nc, make_in_maps(x), core_ids=list(range(N_CORES))
    )
    out = np.concatenate(
        [np.asarray(res.results[k]["out"]) for k in range(N_CORES)], axis=0
    )
    return out.astype(np.float32)


# revision 7
# speedup vs baseline: 1.6809x; 1.0310x over previous
"""3x3 median filter (zero-padded) on TRN2, 8 NeuronCores, bf16 datapath.

Input  x: (32, 3, 512, 512) float32
Output  : (32, 3, 512, 512) float32.

Accuracy: the median network only ever SELECTS one of its 9 inputs (min/max
ops create no new values), so the device-side bf16 result equals the bf16
rounding of the element that is the median of the rounded window. Order
statistics are 1-Lipschitz under sup-norm perturbation, so end-to-end error
is <= 2^-8 relative -- far inside the 2e-2 gate. Measured 3.4e-3.

Strategy
--------
Pure data parallel: batch dim sharded 4-per-core across 8 cores; per core
12 images (4 batch x 3 chan) in 2 groups of 6 images x 2 vertical halves.

bf16 doubles DVE tensor_tensor throughput (2x_1P perf mode) but ONLY for
unit-stride 4-byte-aligned access patterns, so the horizontal stage is
restructured from the fp32 baseline's stride-2 parity tricks into dense
shifted-field form:

  stage 1 (vertical, 5 TT/elem, all dense): row-pair (qmn,qmx) shared by
  both output-row parities, flat over the whole grid.

  stage 2 (horizontal, 12 TT/elem, all dense + aligned): per field
  F in {MN,MD,MX} build s1F[j]=F[j+1] (the ONLY odd-element shift, done
  as a ScalarE copy -- ACT is otherwise idle), then
     P[j]  = op(F[j], s1F[j])          # aligned TT, 2x
     R[j]  = op(P[j], F[j+2])          # +2 elems = 4B-aligned shift, 2x
  R[j] = sliding-3 result centered at col j+1; the final med3(Rmn,Rmd,Rmx)
  writes into an OUT grid whose per-image segment holds col c at position
  c+1, so the wide write starts at even offset 2 and the DMA store (which
  doesn't care about alignment) un-shifts.

  Output cols 0 and 511 (windows containing the zero pad column): ScalarE
  gathers P/Q values at grid positions {0,510} of both parities into one
  dense tile; 6 DVE ops of width 4*nimg + 2 per-parity writes into the
  OUT grids finish them (instead of 14 strided tiny ops).

Grid: per-image segment width 514 (even -> every segment start keeps 4B
parity). Segment positions 512..513 are scratch: stage-1 ops run flat over
the whole grid and compute garbage there; no stored output reads a garbage
lane (out cols 0/511 come from the boundary path).

Image rows 0 and 511 (windows contain the zero pad row): one small
24-partition pass. Its loads are issued up front (tiny); its compute is
issued LAST so it fills the DVE-idle tail while the final block's output
stores drain.

Stage-2 temp tiles alias aggressively (MN1<-Pmn, MX1<-Qmn, TF<-Qmx,
tmd<-Pmx, stage-1 t_o/t_e <- Rmn/Rmd buffers): DVE issue order makes every
WAR safe, and it buys the SBUF headroom for 6-image groups.

Engine budget per core (theory): DVE 17 TT/elem at 2x ~= 220us busy, ACT
~80us, DMA ~19MB. Loads on the SP+ACT HWDGE queues, stores on the GpSimd
SWDGE queue.
"""
import sys

if "/opt/trn_rl_repo" not in sys.path:
    sys.path.insert(0, "/opt/trn_rl_repo")

import numpy as np
import ml_dtypes
import concourse.bacc as bacc
import concourse.mybir as mybir
import concourse.tile as tile
from concourse import bass_utils

B, C, H, W = 32, 3, 512, 512
N_CORES = 8
B_PER = B // N_CORES          # 4 batches per core
NIMG = B_PER * C              # 12 images per core
GIMG = 6                      # images per tile group
PW = W + 2                    # per-image grid width (514, even)
FP = GIMG * PW                # flat grid width of row tiles (3084)
HH = H // 2                   # 256 rows per vertical half
P = 128                       # partitions = row pairs per half
NE = 2 * NIMG                 # partitions in the edge-rows pass (24)

BF16 = mybir.dt.bfloat16
MIN = mybir.AluOpType.min
MAX = mybir.AluOpType.max

_PROGRAM = None


def _seg(T, npart, nimg):
    """[npart, nimg, 514] per-image-segment view."""
    return T[:].rearrange("p (i w) -> p i w", w=PW)[0:npart, 0:nimg]


def _stage2_copies(nc, pm, MN, MD, MX, npart, nimg, pref):
    """ScalarE: the one odd-element shift per field, s1F[j] = F[j+1]."""
    NF = nimg * PW
    s1 = {}
    for name, F in (("MN", MN), ("MD", MD), ("MX", MX)):
        T = pm.tile([P, FP], BF16, tag=f"s1{name}", name=f"{pref}s1{name}")
        nc.scalar.copy(T[:][0:npart, 0 : NF - 1], F[:][0:npart, 1:NF])
        s1[name] = T
    return s1


def _stage2_compute(nc, pm, MN, MD, MX, s1, OUT, npart, nimg, pref, bnd=None):
    """DVE: dense aligned sliding-3 per field + final med3 -> OUT grid.
    OUT per-image position c+1 holds output col c (cols 1..510 here).
    If bnd is given (main blocks), ScalarE gathers the P/Q boundary
    columns into bnd[parity] for the deferred batched boundary pass;
    otherwise (edge pass) boundary cols are done inline."""
    NF = nimg * PW
    NI = NF - 2
    fl = lambda T, a, b: T[:][0:npart, a:b]

    def t2(tag):
        return pm.tile([P, FP], BF16, tag=tag, name=f"{pref}{tag}")

    Pmn, Pmx, Qmn, Qmx = t2("Pmn"), t2("Pmx"), t2("Qmn"), t2("Qmx")
    Rmn, Rmd, Rmx = t2("Rmn"), t2("Rmd"), t2("Rmx")
    # aliases -- disjoint lifetimes given the op order below
    tmd = pm.tile([P, FP], BF16, tag="Pmx", name=f"{pref}tmd")
    MN1 = pm.tile([P, FP], BF16, tag="Pmn", name=f"{pref}MN1")
    MX1 = pm.tile([P, FP], BF16, tag="Qmn", name=f"{pref}MX1")
    TF = pm.tile([P, FP], BF16, tag="Qmx", name=f"{pref}TF")

    tt = nc.vector.tensor_tensor
    # adjacent-column pairs (j, j+1)
    tt(fl(Pmn, 0, NI), fl(MN, 0, NI), fl(s1["MN"], 0, NI), op=MAX)
    tt(fl(Qmn, 0, NI), fl(MD, 0, NI), fl(s1["MD"], 0, NI), op=MIN)
    tt(fl(Qmx, 0, NI), fl(MD, 0, NI), fl(s1["MD"], 0, NI), op=MAX)
    tt(fl(Pmx, 0, NI), fl(MX, 0, NI), fl(s1["MX"], 0, NI), op=MIN)

    gv = lambda T: _seg(T, npart, nimg)[:, :, 0:511:510]
    if bnd is not None:
        # ScalarE pulls the {0,510} boundary columns out now so the P/Q
        # buffers can be reused (aliases above) and the boundary math can
        # run batched across both parities later
        for nm, T in (("Pmn", Pmn), ("Pmx", Pmx), ("Qmn", Qmn), ("Qmx", Qmx)):
            nc.scalar.copy(bnd[nm], gv(T))

    # close the window with the +2 (aligned) shift: R[j] ~ output col j+1
    tt(fl(Rmn, 0, NI), fl(Pmn, 0, NI), fl(MN, 2, NF), op=MAX)
    tt(fl(Rmx, 0, NI), fl(Pmx, 0, NI), fl(MX, 2, NF), op=MIN)
    tt(fl(tmd, 0, NI), fl(Qmx, 0, NI), fl(MD, 2, NF), op=MIN)
    tt(fl(Rmd, 0, NI), fl(Qmn, 0, NI), fl(tmd, 0, NI), op=MAX)
    # med3(Rmn, Rmd, Rmx); last op writes output cols 1..510 at grid
    # positions 2..511 (even start -> stays in 2x mode)
    tt(fl(MN1, 0, NI), fl(Rmn, 0, NI), fl(Rmd, 0, NI), op=MIN)
    tt(fl(MX1, 0, NI), fl(Rmn, 0, NI), fl(Rmd, 0, NI), op=MAX)
    tt(fl(TF, 0, NI), fl(MX1, 0, NI), fl(Rmx, 0, NI), op=MIN)
    ov = _seg(OUT, npart, nimg)[:, :, 2:512]
    tt(ov, _seg(MN1, npart, nimg)[:, :, 0:510],
       _seg(TF, npart, nimg)[:, :, 0:510], op=MAX)

    if bnd is None:
        # inline boundary (edge pass only): A=max(P,0), C=min(Pmx,0),
        # B=max(Qmn,min(Qmx,0)), med3 -> OUT positions {1, 512}
        bt = lambda tag: pm.tile([P, 2 * GIMG], BF16, tag=f"{tag}b",
                                 name=f"{pref}{tag}b")
        bv = lambda T: T[:].rearrange("p (i c) -> p i c", c=2)[0:npart, 0:nimg]
        BA, BB, BC = bt("BA"), bt("BB"), bt("BC")
        B1, B2, B3 = bt("B1"), bt("B2"), bt("B3")
        nc.vector.tensor_scalar_max(bv(BA), gv(Pmn), 0.0)
        nc.vector.tensor_scalar_min(bv(BC), gv(Pmx), 0.0)
        nc.vector.scalar_tensor_tensor(bv(BB), gv(Qmx), 0.0, gv(Qmn),
                                       op0=MIN, op1=MAX)
        tt(bv(B1), bv(BA), bv(BB), op=MIN)
        tt(bv(B2), bv(BA), bv(BB), op=MAX)
        tt(bv(B3), bv(B2), bv(BC), op=MIN)
        obv = _seg(OUT, npart, nimg)[:, :, 1:513:511]
        tt(obv, bv(B1), bv(B3), op=MAX)


def _boundary_batch(nc, pm, BP, OUT_o, OUT_e):
    """Output cols 0 and 511 for both parities in one dense pass.
    BP[nm] tiles are [P, 2*GIMG*2] with layout (parity, img, col)."""
    tt = nc.vector.tensor_tensor
    bt = lambda tag: pm.tile([P, 4 * GIMG], BF16, tag=f"{tag}b", name=f"{tag}b")
    BA, BB, BC = bt("BA"), bt("BB"), bt("BC")
    B1, B2, B3 = bt("B1"), bt("B2"), bt("B3")
    nc.vector.tensor_scalar_max(BA[:], BP["Pmn"][:], 0.0)
    nc.vector.tensor_scalar_min(BC[:], BP["Pmx"][:], 0.0)
    nc.vector.scalar_tensor_tensor(BB[:], BP["Qmx"][:], 0.0, BP["Qmn"][:],
                                   op0=MIN, op1=MAX)
    tt(B1[:], BA[:], BB[:], op=MIN)
    tt(B2[:], BA[:], BB[:], op=MAX)
    tt(B3[:], B2[:], BC[:], op=MIN)
    pv = lambda T, h: T[:].rearrange("p (h i c) -> p h i c", h=2, c=2)[:, h]
    for h, OUT in ((0, OUT_o), (1, OUT_e)):
        obv = _seg(OUT, P, GIMG)[:, :, 1:513:511]
        tt(obv, pv(B1, h), pv(B3, h), op=MAX)


def _block(nc, pio, pm, xh, oh, g, half):
    """One vertical half of one image group: odd output rows r0+1..r0+255,
    even rows r0+2..r0+256 (halves overlap by two rows so every DMA is a
    full 128-partition transfer). Rows 0 and 511 via the edge pass."""
    r0 = 0 if half == 0 else H - HH - 2
    i0 = GIMG * g

    E = pio.tile([P, FP], BF16, tag="E", name="E", bufs=2)
    O = pio.tile([P, FP], BF16, tag="O", name="O", bufs=2)
    E_sh = pio.tile([P, FP], BF16, tag="E_sh", name="E_sh", bufs=2)
    O_sh2 = pio.tile([P, FP], BF16, tag="O_sh2", name="O_sh2", bufs=2)

    # scratch cols 512..513 of each segment are read by the flat stage-1
    # ops but never loaded: define them so no lane is uninitialized
    for T in (E, O, E_sh, O_sh2):
        nc.gpsimd.memset(_seg(T, P, GIMG)[:, :, W:PW], 0.0)

    lv = lambda T: _seg(T, P, GIMG)[:, :, 0:W]
    img = lambda r_lo: xh[r_lo : min(r_lo + 2 * P, H) : 2, i0 : i0 + GIMG, :]
    # queue order matters (HWDGE queues are FIFOs): the (O, E_sh) pair
    # feeds the first op of the block, so those loads go first
    nc.sync.dma_start(lv(E_sh), img(r0 + 2))     # rows r0+2p+2
    nc.scalar.dma_start(lv(O), img(r0 + 1))      # rows r0+2p+1
    nc.sync.dma_start(lv(E), img(r0))            # rows r0+2p
    nc.scalar.dma_start(lv(O_sh2), img(r0 + 3))  # rows r0+2p+3

    # stage 1: shared pair = (O, E_sh) = rows (2p+1, 2p+2); flat ops
    qmn = pm.tile([P, FP], BF16, tag="qmn", name="qmn")
    qmx = pm.tile([P, FP], BF16, tag="qmx", name="qmx")
    tt = nc.vector.tensor_tensor
    tt(qmn[:], O[:], E_sh[:], op=MIN)
    tt(qmx[:], O[:], E_sh[:], op=MAX)

    def fld(tag):
        return pm.tile([P, FP], BF16, tag=tag, name=tag)

    MN_o, MD_o, MX_o = fld("MN_o"), fld("MD_o"), fld("MX_o")
    MN_e, MD_e, MX_e = fld("MN_e"), fld("MD_e"), fld("MX_e")
    # stage-1 temps alias stage-2 R slots (dead before those are written)
    t_o = pm.tile([P, FP], BF16, tag="Rmn", name="t_o")
    t_e = pm.tile([P, FP], BF16, tag="Rmd", name="t_e")

    # odd output rows r0+2p+1: pair + E (row r0+2p)
    tt(MN_o[:], qmn[:], E[:], op=MIN)
    tt(MX_o[:], qmx[:], E[:], op=MAX)
    tt(t_o[:], qmx[:], E[:], op=MIN)
    tt(MD_o[:], qmn[:], t_o[:], op=MAX)
    # even output rows r0+2p+2: pair + O_sh2 (row r0+2p+3)
    tt(MN_e[:], qmn[:], O_sh2[:], op=MIN)
    tt(MX_e[:], qmx[:], O_sh2[:], op=MAX)
    tt(t_e[:], qmx[:], O_sh2[:], op=MIN)
    tt(MD_e[:], qmn[:], t_e[:], op=MAX)

    # boundary gather tiles: layout (parity, img, col{0,510})
    BP = {
        nm: pm.tile([P, 4 * GIMG], BF16, tag=f"BP{nm}", name=f"BP{nm}")
        for nm in ("Pmn", "Pmx", "Qmn", "Qmx")
    }
    hv = lambda nm, h: BP[nm][:].rearrange("p (h i c) -> p h i c", h=2, c=2)[:, h]

    OUT_o = pio.tile([P, FP], BF16, tag="OUT_o", name="OUT_o")
    OUT_e = pio.tile([P, FP], BF16, tag="OUT_e", name="OUT_e")
    s1_o = _stage2_copies(nc, pm, MN_o, MD_o, MX_o, P, GIMG, "o_")
    _stage2_compute(nc, pm, MN_o, MD_o, MX_o, s1_o, OUT_o, P, GIMG, "o_",
                    bnd={nm: hv(nm, 0) for nm in BP})
    s1_e = _stage2_copies(nc, pm, MN_e, MD_e, MX_e, P, GIMG, "e_")
    _stage2_compute(nc, pm, MN_e, MD_e, MX_e, s1_e, OUT_e, P, GIMG, "e_",
                    bnd={nm: hv(nm, 1) for nm in BP})
    _boundary_batch(nc, pm, BP, OUT_o, OUT_e)

    out_img = lambda r_lo: oh[r_lo : min(r_lo + 2 * P, H) : 2, i0 : i0 + GIMG, :]
    ostore = lambda T: _seg(T, P, GIMG)[:, :, 1:513]
    # stores on the SWDGE queue so they never block a later block's loads
    nc.gpsimd.dma_start(out_img(r0 + 1), ostore(OUT_o))
    nc.gpsimd.dma_start(out_img(r0 + 2), ostore(OUT_e))


def _edge_loads(nc, pio, xi):
    """Loads for image rows 0 and 511 (tiny, partial-partition): issued up
    front so the end-of-kernel edge compute never waits on DMA."""
    R0 = pio.tile([NE, PW], BF16, tag="R0", name="R0")
    R1 = pio.tile([NE, PW], BF16, tag="R1", name="R1")
    for T in (R0, R1):
        nc.gpsimd.memset(T[:][0:NE, W:PW], 0.0)
    nc.sync.dma_start(R0[:][0:NIMG, 0:W], xi[:, 0, :])
    nc.scalar.dma_start(R1[:][0:NIMG, 0:W], xi[:, 1, :])
    nc.sync.dma_start(R0[:][NIMG:NE, 0:W], xi[:, H - 1, :])
    nc.scalar.dma_start(R1[:][NIMG:NE, 0:W], xi[:, H - 2, :])
    return R0, R1


def _edge_compute(nc, pio, pm, oi, R0, R1):
    """Rows 0 and 511 (windows contain the zero pad row), 24 partitions:
    p 0..11 = row 0 of image p; p 12..23 = row 511 of image p-12. Runs
    last, in the shadow of the final block's output stores."""
    rmn = pm.tile([NE, PW], BF16, tag="rmn", name="rmn")
    rmx = pm.tile([NE, PW], BF16, tag="rmx", name="rmx")
    nc.vector.tensor_tensor(rmn[:], R0[:], R1[:], op=MIN)
    nc.vector.tensor_tensor(rmx[:], R0[:], R1[:], op=MAX)

    # vertical sort3 with the zero pad row: min/max vs 0, med=max(mn,min(mx,0))
    MN0 = pm.tile([NE, PW], BF16, tag="eMN", name="eMN")
    MD0 = pm.tile([NE, PW], BF16, tag="eMD", name="eMD")
    MX0 = pm.tile([NE, PW], BF16, tag="eMX", name="eMX")
    nc.vector.tensor_scalar_min(MN0[:], rmn[:], 0.0)
    nc.vector.tensor_scalar_max(MX0[:], rmx[:], 0.0)
    nc.vector.scalar_tensor_tensor(MD0[:], rmx[:], 0.0, rmn[:], op0=MIN, op1=MAX)

    s1 = {}
    for name, F in (("MN", MN0), ("MD", MD0), ("MX", MX0)):
        T = pm.tile([NE, PW], BF16, tag=f"es1{name}", name=f"es1{name}")
        nc.scalar.copy(T[:][0:NE, 0 : PW - 1], F[:][0:NE, 1:PW])
        s1[name] = T

    OUT0 = pio.tile([NE, PW], BF16, tag="OUT0", name="OUT0")
    _stage2_compute_small(nc, pm, MN0, MD0, MX0, s1, OUT0)
    nc.gpsimd.dma_start(oi[:, 0, :], OUT0[:][0:NIMG, 1:513])
    nc.gpsimd.dma_start(oi[:, H - 1, :], OUT0[:][NIMG:NE, 1:513])


def _stage2_compute_small(nc, pm, MN, MD, MX, s1, OUT):
    """Same dataflow as _stage2_compute on [NE, PW] tiles (nimg=1)."""
    NF = PW
    NI = NF - 2
    fl = lambda T, a, b: T[:][0:NE, a:b]

    def t2(tag):
        return pm.tile([NE, PW], BF16, tag=f"e{tag}", name=f"e{tag}")

    Pmn, Pmx, Qmn, Qmx = t2("Pmn"), t2("Pmx"), t2("Qmn"), t2("Qmx")
    tmd, Rmn, Rmd, Rmx = t2("tmd"), t2("Rmn"), t2("Rmd"), t2("Rmx")
    MN1, MX1, TF = t2("MN1"), t2("MX1"), t2("TF")

    tt = nc.vector.tensor_tensor
    tt(fl(Pmn, 0, NI), fl(MN, 0, NI), fl(s1["MN"], 0, NI), op=MAX)
    tt(fl(Qmn, 0, NI), fl(MD, 0, NI), fl(s1["MD"], 0, NI), op=MIN)
    tt(fl(Qmx, 0, NI), fl(MD, 0, NI), fl(s1["MD"], 0, NI), op=MAX)
    tt(fl(Pmx, 0, NI), fl(MX, 0, NI), fl(s1["MX"], 0, NI), op=MIN)
    tt(fl(Rmn, 0, NI), fl(Pmn, 0, NI), fl(MN, 2, NF), op=MAX)
    tt(fl(tmd, 0, NI), fl(Qmx, 0, NI), fl(MD, 2, NF), op=MIN)
    tt(fl(Rmd, 0, NI), fl(Qmn, 0, NI), fl(tmd, 0, NI), op=MAX)
    tt(fl(Rmx, 0, NI), fl(Pmx, 0, NI), fl(MX, 2, NF), op=MIN)
    tt(fl(MN1, 0, NI), fl(Rmn, 0, NI), fl(Rmd, 0, NI), op=MIN)
    tt(fl(MX1, 0, NI), fl(Rmn, 0, NI), fl(Rmd, 0, NI), op=MAX)
    tt(fl(TF, 0, NI), fl(MX1, 0, NI), fl(Rmx, 0, NI), op=MIN)
    tt(fl(OUT, 2, 512), fl(MN1, 0, 510), fl(TF, 0, 510), op=MAX)

    gv = lambda T: T[:][0:NE, 0:511:510]
    bt = lambda tag: pm.tile([NE, 2], BF16, tag=f"e{tag}b", name=f"e{tag}b")
    BA, BB, BC = bt("BA"), bt("BB"), bt("BC")
    B1, B2, B3 = bt("B1"), bt("B2"), bt("B3")
    nc.vector.tensor_scalar_max(BA[:], gv(Pmn), 0.0)
    nc.vector.tensor_scalar_min(BC[:], gv(Pmx), 0.0)
    nc.vector.scalar_tensor_tensor(BB[:], gv(Qmx), 0.0, gv(Qmn), op0=MIN, op1=MAX)
    tt(B1[:], BA[:], BB[:], op=MIN)
    tt(B2[:], BA[:], BB[:], op=MAX)
    tt(B3[:], B2[:], BC[:], op=MIN)
    tt(OUT[:][0:NE, 1:513:511], B1[:], B3[:], op=MAX)


def build_program():
    nc = bacc.Bacc(
        "TRN2", target_bir_lowering=False, debug=False, num_devices=N_CORES
    )
    x_d = nc.dram_tensor("x", [B_PER, C, H, W], BF16, kind="ExternalInput").ap()
    o_d = nc.dram_tensor("out", [B_PER, C, H, W], BF16, kind="ExternalOutput").ap()
    xh = x_d.rearrange("b c h w -> h (b c) w")  # [512, 12, 512]
    oh = o_d.rearrange("b c h w -> h (b c) w")
    xi = x_d.rearrange("b c h w -> (b c) h w")  # [12, 512, 512]
    oi = o_d.rearrange("b c h w -> (b c) h w")

    with tile.TileContext(nc) as tc:
        with (
            tc.tile_pool(name="io", bufs=1) as pio,
            tc.tile_pool(name="mid", bufs=1) as pm,
        ):
            _block(nc, pio, pm, xh, oh, 0, 0)
            R0, R1 = _edge_loads(nc, pio, xi)
            _block(nc, pio, pm, xh, oh, 0, 1)
            for g in range(1, NIMG // GIMG):
                for half in range(2):
                    _block(nc, pio, pm, xh, oh, g, half)
            _edge_compute(nc, pio, pm, oi, R0, R1)
    nc.compile()
    return nc


def _get_program():
    global _PROGRAM
    if _PROGRAM is None:
        _PROGRAM = build_program()
    return _PROGRAM


def make_in_maps(x: np.ndarray):
    xb = np.ascontiguousarray(x).astype(ml_dtypes.bfloat16)
    return [{"x": xb[k * B_PER : (k + 1) * B_PER]} for k in range(N_CORES)]


def kernel(**inputs) -> np.ndarray:
    x = np.asarray(inputs["x"], dtype=np.float32)
    assert x.shape == (B, C, H, W), x.shape
    nc = _get_program()
    res = bass_utils.run_bass_kernel_spmd(
        nc, make_in_maps(x), core_ids=list(range(N_CORES))
    )
    out = np.concatenate(
        [np.asarray(res.results[k]["out"]) for k in range(N_CORES)], axis=0
    )
    return out.astype(np.float32)
